# revision 6
# baseline (speedup 1.0000x reference)
"""Trainium2 Bass kernel for nn_CrossEntropyLoss_22419729285187.

Computes  -sum_{matched, non-BG true rows} dot(y_true[i,1:], y_pred[rank_i]) / count
sharded over 8 NeuronCores.

Strategy (per sharding hint): the host performs the cheap key join
(encode + searchsorted) and compacts to the matched AND non-background
(true,pred) row pairs — the r-th matched true row pairs positionally
with y_pred_features[r], so dropping BG rows keeps the pairing and the
count k is known on host. The pairs are cast to fp16 (the final scalar
tolerates ~3e-4 rel err; accumulation stays fp32 on device) and packed
row-wise into one [rows, 64] tensor ([yt_row | yp_row]) so each tile is
a single large-chunk DMA (16KB+ contiguous per partition). Shards are
row-distributed across the 8 cores; each core streams ~8.3MB through
tiles alternating between the two HWDGE queues and runs one fused
multiply-reduce per tile on the DVE (strided in0/in1 views of the
packed tile, fp32 accumulate). Tile sizes descend so the last tile's
compute tail is short; each tile's [128, 1] partial is written out
right after its STT (the SBUF->DRAM queue is warmed early) so the
end-of-program drain isn't stuck behind a cold write queue. Host sums
the [128, n_tiles] partials from all cores for the final -num/k.
"""

import os
import sys

for _p in ("/opt/trn_rl_repo", "/root/.axon_site/_ro/trn_rl_repo"):
    if os.path.isdir(_p) and _p not in sys.path:
        sys.path.append(_p)

import numpy as np

N_CORES = 8
PARTS = 128

# Rows per partition per tile. DMA queues drain tile loads FIFO, so:
# small first tiles let the DVE start within ~1us of streaming; big
# middle tiles keep DMA descriptor chunks large; small last tiles
# shorten the final load->STT->out dependency chain.
TILE_GS = (8, 16, 126, 126, 126, 77, 16, 8)  # sum = 503 = ceil(64283/128)

_compiled = {}
_last_results = None


def _encode(idx):
    idx = idx.astype(np.int64)
    return ((idx[:, 0] * 1024 + idx[:, 1]) * 1024 + idx[:, 2]) * 1024 + idx[:, 3]


def _build_program(gs, c_pred):
    """Build + schedule the SPMD Tile program for one core shard.

    The shard's rows are laid out tile-by-tile, [partition][group]
    within each tile: tile t holds PARTS*gs[t] rows, each partition
    owning gs[t] consecutive packed rows (contiguous gs[t]*2*c_pred
    fp16 run: [yt | yp] per row).
    """
    from concourse import bacc
    import concourse.mybir as mybir
    from concourse.tile import TileContext

    f16 = mybir.dt.float16
    f32 = mybir.dt.float32
    nt = len(gs)
    w = 2 * c_pred
    r_pad = PARTS * sum(gs)

    nc = bacc.Bacc("TRN2", target_bir_lowering=False, debug=False,
                   num_devices=N_CORES)
    x_d = nc.dram_tensor("x", [r_pad, w], f16, kind="ExternalInput")
    out_d = nc.dram_tensor("partials", [PARTS, nt], f32, kind="ExternalOutput")
    warm_d = nc.dram_tensor("warm", [1, 1], f32, kind="ExternalOutput")

    qs = [nc.sync, nc.scalar]
    with TileContext(nc) as tc:
        with tc.tile_pool(name="acc", bufs=1) as accp:
            red_all = accp.tile([PARTS, nt], f32)
            warm = accp.tile([1, 1], f32)
            # bufs=nt: every tile load triggers up-front so the DMA
            # engines never starve behind buffer reuse.
            with tc.tile_pool(name="io", bufs=nt) as pool, \
                 tc.tile_pool(name="scrp", bufs=2) as scrp:
                row0 = 0
                for t, g in enumerate(gs):
                    seg = PARTS * g
                    x_v = x_d.ap()[row0:row0 + seg, :].rearrange(
                        "(p g) c -> p g c", p=PARTS, g=g)
                    row0 += seg
                    x_t = pool.tile([PARTS, g, w], f16, tag="x")
                    qs[t % 2].dma_start(out=x_t[:, :, :], in_=x_v)
                    if t == 0:
                        # Touch the SBUF->DRAM write path before it
                        # matters (cold-queue completion costs ~7us),
                        # after the first loads are already triggered.
                        nc.vector.memset(warm[:], 0.0)
                        nc.sync.dma_start(out=warm_d[:, :], in_=warm[:])
                    scr = scrp.tile([PARTS, g, c_pred], f16, tag="scr")
                    # red_all[:, t] = sum_{g,c} yt * yp  (fp32 accum)
                    nc.vector.scalar_tensor_tensor(
                        out=scr[:, :, :], in0=x_t[:, :, 0:c_pred], scalar=1.0,
                        in1=x_t[:, :, c_pred:w],
                        op0=mybir.AluOpType.mult, op1=mybir.AluOpType.mult,
                        accum_out=red_all[:, t:t + 1])
            # One contiguous-ish [128, nt] write: column writes cost 128
            # scattered 4B descriptors each; batching them 6x fewer.
            nc.sync.dma_start(out=out_d[:, :], in_=red_all[:])
    nc.compile()
    return nc


def kernel(y_true_features, y_true_indices, y_pred_features, y_pred_indices):
    global _last_results
    from concourse.bass_utils import run_bass_kernel_spmd

    yt = np.ascontiguousarray(np.asarray(y_true_features, dtype=np.float32))
    yp = np.ascontiguousarray(np.asarray(y_pred_features, dtype=np.float32))
    n, c1 = yt.shape
    m, c = yp.shape

    # ---- host-side key join (cheap integer work) ----
    kt = _encode(np.asarray(y_true_indices))
    kp = _encode(np.asarray(y_pred_indices))
    kps = np.sort(kp)
    pos = np.clip(np.searchsorted(kps, kt), 0, m - 1)
    matched = kps[pos] == kt
    # Only matched non-BG true rows contribute. The r-th matched true
    # row (row order) pairs with y_pred_features[r] positionally (rank
    # = cumsum(matched)-1 is sequential over matched rows). Dropping BG
    # rows from both sides keeps the pairing; k is then known here.
    midx = np.flatnonzero(matched)
    nb = yt[midx, 0] != 1.0                      # non-BG mask over matched rows
    k = int(nb.sum())
    A = yt[midx[nb], 1:].astype(np.float16)      # [k, c]
    B = yp[:midx.size][nb].astype(np.float16)    # [k, c]

    # ---- shard the k contributing pairs across cores ----
    rows = -(-k // N_CORES)
    gsum = -(-rows // PARTS)
    gs = list(TILE_GS)
    if sum(gs) < gsum:  # grow the big tiles if the shard is larger
        gs[0] += gsum - sum(gs)
    else:               # shrink from the front to fit
        over = sum(gs) - gsum
        for i in range(len(gs)):
            take = min(over, gs[i] - 1)
            gs[i] -= take
            over -= take
        gs = [g for g in gs if g > 0]
    gs = tuple(gs)
    r_pad = PARTS * sum(gs)

    key = (gs, c)
    if key not in _compiled:
        _compiled[key] = _build_program(gs, c)
    nc = _compiled[key]

    in_maps = []
    for i in range(N_CORES):
        lo, hi = i * rows, min((i + 1) * rows, k)
        nr = max(hi - lo, 0)
        x_c = np.zeros((r_pad, 2 * c), dtype=np.float16)
        x_c[:nr, :c] = A[lo:hi]
        x_c[:nr, c:] = B[lo:hi]
        in_maps.append({"x": x_c})

    res = run_bass_kernel_spmd(nc, in_maps, list(range(N_CORES)))
    _last_results = res

    num = 0.0
    for i in range(N_CORES):
        p = res.results[i]["partials"]
        num += float(p.sum(dtype=np.float64))
    return np.float32(-num / k)


# revision 7
# speedup vs baseline: 1.1016x; 1.1016x over previous
"""Trainium2 Bass kernel for nn_CrossEntropyLoss_22419729285187.

Computes  -sum_{matched, non-BG true rows} dot(y_true[i,1:], y_pred[rank_i]) / count
sharded over 8 NeuronCores.

Strategy (per sharding hint): the host performs the cheap key join
(encode + searchsorted) and compacts to the matched AND non-background
(true,pred) row pairs — the r-th matched true row pairs positionally
with y_pred_features[r], so dropping BG rows keeps the pairing and the
count k is known on host. The pairs are cast to fp16 (the final scalar
tolerates ~3e-4 rel err; accumulation stays fp32 on device) and packed
row-wise into one [rows, 64] tensor ([yt_row | yp_row]) so each tile is
a single large-chunk DMA (16KB+ contiguous per partition). Shards are
row-distributed across the 8 cores; each core streams ~8.3MB through
tiles alternating between the two HWDGE queues and runs one fused
multiply-reduce per tile on the DVE (strided in0/in1 views of the
packed tile, fp32 accumulate). Tile sizes descend so the last tile's
compute tail is short; each tile's [128, 1] partial is written out
right after its STT (the SBUF->DRAM queue is warmed early) so the
end-of-program drain isn't stuck behind a cold write queue. Host sums
the [128, n_tiles] partials from all cores for the final -num/k.
"""

import os
import sys

for _p in ("/opt/trn_rl_repo", "/root/.axon_site/_ro/trn_rl_repo"):
    if os.path.isdir(_p) and _p not in sys.path:
        sys.path.append(_p)

import numpy as np

N_CORES = 8
PARTS = 128

# Rows per partition per tile. DMA queues drain tile loads FIFO, so:
# small first tiles let the DVE start within ~1us of streaming; big
# middle tiles keep DMA descriptor chunks large; small last tiles
# shorten the final load->STT->out dependency chain.
TILE_GS = (16, 126, 126, 126, 93, 16)  # sum = 503 = ceil(64283/128)

_compiled = {}
_last_results = None


def _encode(idx):
    idx = idx.astype(np.int64)
    return ((idx[:, 0] * 1024 + idx[:, 1]) * 1024 + idx[:, 2]) * 1024 + idx[:, 3]


def _build_program(gs, c_pred):
    """Build + schedule the SPMD Tile program for one core shard.

    The shard's rows are laid out tile-by-tile, [partition][group]
    within each tile: tile t holds PARTS*gs[t] rows, each partition
    owning gs[t] consecutive packed rows (contiguous gs[t]*2*c_pred
    fp16 run: [yt | yp] per row).
    """
    from concourse import bacc
    import concourse.mybir as mybir
    from concourse.tile import TileContext

    f16 = mybir.dt.float16
    f32 = mybir.dt.float32
    nt = len(gs)
    w = 2 * c_pred
    r_pad = PARTS * sum(gs)

    nc = bacc.Bacc("TRN2", target_bir_lowering=False, debug=False,
                   num_devices=N_CORES)
    x_d = nc.dram_tensor("x", [r_pad, w], f16, kind="ExternalInput")
    out_d = nc.dram_tensor("partials", [PARTS, nt], f32, kind="ExternalOutput")
    warm_d = nc.dram_tensor("warm", [1, 1], f32, kind="ExternalOutput")

    qs = [nc.sync, nc.scalar]
    with TileContext(nc) as tc:
        with tc.tile_pool(name="acc", bufs=1) as accp:
            red_all = accp.tile([PARTS, nt], f32)
            warm = accp.tile([1, 1], f32)
            # bufs=nt: every tile load triggers up-front so the DMA
            # engines never starve behind buffer reuse.
            with tc.tile_pool(name="io", bufs=nt) as pool, \
                 tc.tile_pool(name="scrp", bufs=2) as scrp:
                row0 = 0
                for t, g in enumerate(gs):
                    seg = PARTS * g
                    x_v = x_d.ap()[row0:row0 + seg, :].rearrange(
                        "(p g) c -> p g c", p=PARTS, g=g)
                    row0 += seg
                    x_t = pool.tile([PARTS, g, w], f16, tag="x")
                    qs[t % 2].dma_start(out=x_t[:, :, :], in_=x_v)
                    if t == 0:
                        # Touch the SBUF->DRAM write path before it
                        # matters (cold-queue completion costs ~7us),
                        # after the first loads are already triggered.
                        nc.vector.memset(warm[:], 0.0)
                        nc.sync.dma_start(out=warm_d[:, :], in_=warm[:])
                    scr = scrp.tile([PARTS, g, c_pred], f16, tag="scr")
                    # red_all[:, t] = sum_{g,c} yt * yp  (fp32 accum)
                    nc.vector.scalar_tensor_tensor(
                        out=scr[:, :, :], in0=x_t[:, :, 0:c_pred], scalar=1.0,
                        in1=x_t[:, :, c_pred:w],
                        op0=mybir.AluOpType.mult, op1=mybir.AluOpType.mult,
                        accum_out=red_all[:, t:t + 1])
            # One contiguous-ish [128, nt] write: column writes cost 128
            # scattered 4B descriptors each; batching them 6x fewer.
            nc.sync.dma_start(out=out_d[:, :], in_=red_all[:])
    nc.compile()
    return nc


def kernel(y_true_features, y_true_indices, y_pred_features, y_pred_indices):
    global _last_results
    from concourse.bass_utils import run_bass_kernel_spmd

    yt = np.ascontiguousarray(np.asarray(y_true_features, dtype=np.float32))
    yp = np.ascontiguousarray(np.asarray(y_pred_features, dtype=np.float32))
    n, c1 = yt.shape
    m, c = yp.shape

    # ---- host-side key join (cheap integer work) ----
    kt = _encode(np.asarray(y_true_indices))
    kp = _encode(np.asarray(y_pred_indices))
    kps = np.sort(kp)
    pos = np.clip(np.searchsorted(kps, kt), 0, m - 1)
    matched = kps[pos] == kt
    # Only matched non-BG true rows contribute. The r-th matched true
    # row (row order) pairs with y_pred_features[r] positionally (rank
    # = cumsum(matched)-1 is sequential over matched rows). Dropping BG
    # rows from both sides keeps the pairing; k is then known here.
    midx = np.flatnonzero(matched)
    nb = yt[midx, 0] != 1.0                      # non-BG mask over matched rows
    k = int(nb.sum())
    A = yt[midx[nb], 1:].astype(np.float16)      # [k, c]
    B = yp[:midx.size][nb].astype(np.float16)    # [k, c]

    # ---- shard the k contributing pairs across cores ----
    rows = -(-k // N_CORES)
    gsum = -(-rows // PARTS)
    gs = list(TILE_GS)
    if sum(gs) < gsum:  # grow the big tiles if the shard is larger
        gs[0] += gsum - sum(gs)
    else:               # shrink from the front to fit
        over = sum(gs) - gsum
        for i in range(len(gs)):
            take = min(over, gs[i] - 1)
            gs[i] -= take
            over -= take
        gs = [g for g in gs if g > 0]
    gs = tuple(gs)
    r_pad = PARTS * sum(gs)

    key = (gs, c)
    if key not in _compiled:
        _compiled[key] = _build_program(gs, c)
    nc = _compiled[key]

    in_maps = []
    for i in range(N_CORES):
        lo, hi = i * rows, min((i + 1) * rows, k)
        nr = max(hi - lo, 0)
        x_c = np.zeros((r_pad, 2 * c), dtype=np.float16)
        x_c[:nr, :c] = A[lo:hi]
        x_c[:nr, c:] = B[lo:hi]
        in_maps.append({"x": x_c})

    res = run_bass_kernel_spmd(nc, in_maps, list(range(N_CORES)))
    _last_results = res

    num = 0.0
    for i in range(N_CORES):
        p = res.results[i]["partials"]
        num += float(p.sum(dtype=np.float64))
    return np.float32(-num / k)
